# revision 4
# baseline (speedup 1.0000x reference)
"""CPRLinear Trainium2 kernel (v2).

y = x[:, col_indices] @ W_deq.T + bias, where W_deq is the per-128-column-tile
affine dequantization of [W_high_q | W_low_q] (int codes, values 0..63).

Sharding: out_features (8192) split across 8 NeuronCores, 1024 rows each.
x / col_indices replicated (x shipped transposed in bf16 so the column
permutation becomes a contiguous row gather on device).

v2 changes vs v1 (212-249us):
  - W codes shipped as int8 (4x less HBM traffic: 32MB -> 8MB per core);
    high/low slabs concatenated on host so chunks never straddle a boundary
  - xT shipped as bf16 (8MB -> 4MB), gathered with ONE batched dma_gather
    per k-chunk (128*tpc rows/instr) instead of 64 per-tile indirect DMAs
    (SWDGE prep: 66us -> ~7us of Pool time)
  - dequant split across DVE (tensor_scalar) and ACT (activation with
    per-partition scale/bias): both engines ~60% / 40% of the 512 tiles
  - leading 512-k chunk primes the pipeline before the steady 2048-k chunks

Per-core device pipeline (chunks of k):
  - x: dma_gather row-gather of bf16 xT rows by wrapped col_indices
    -> xb tiles [128k, t, 256b]
  - W: HWDGE int8 loads (natural [o,k] row slabs), dequant (q - z) * s with
    per-partition scale/zero columns into per-half staging tiles
    [128o, (t, ob-in-half, j)], then ONE xbar DMA-transpose per half
    (ACT ring only; cross-ring split corrupts on HW). Transposed blocks land
    at wt[k, oc, t, obh, o] so each matmul rhs is one contiguous 512-run
  - TensorE: y[b,o] accumulated over 64 k-tiles in 4 PSUM groups
    (2 b-blocks x 2 o-halves, N=512), bias folded in via a ones-row matmul
  - DVE evacuates PSUM -> SBUF per half as its group closes, HWDGE stores
    y slab [256, 1024] f32; host concatenates slabs along out_features
"""

import os
import sys

import numpy as np

for _p in ("/root/.axon_site", "/root/.axon_site/_ro/trn_rl_repo",
           "/root/.axon_site/_ro/pypackages", "/opt/trn_rl_repo"):
    if os.path.isdir(_p) and _p not in sys.path:
        sys.path.append(_p)

B, IN, OUT = 256, 8192, 8192
N_CORES = 8
O_SLAB = OUT // N_CORES          # 1024 out rows per core
N_HIGH, N_LOW = 2048, 6144
TILE = 128
NT = IN // TILE                  # 64 k-tiles
OB = O_SLAB // TILE              # 8 o-blocks per core
# (k_offset, k_len) chunks; small leading chunk shortens time to first matmul
CHUNK_PLAN = [(0, 512), (512, 1536), (2048, 2048), (4096, 2048), (6144, 2048)]
# fraction of dequant tiles on DVE (rest on ACT); DVE ~194ns/tile vs ACT ~292
TS_DVE_SHARE = float(os.environ.get("KERNEL_TS_DVE_SHARE", "0.625"))

_PROGRAM = None


def _build_program(n_bodies=1):
    import concourse.bass as bass
    import concourse.bacc as bacc
    import concourse.tile as tile
    import concourse.mybir as mybir

    f32 = mybir.dt.float32
    bf16 = mybir.dt.bfloat16
    i8 = mybir.dt.int8
    i16 = mybir.dt.int16

    nc = bacc.Bacc(
        "TRN2",
        target_bir_lowering=False,
        debug=False,
        enable_asserts=False,
        num_devices=N_CORES,
    )

    i32 = mybir.dt.int32
    xT = nc.dram_tensor("xT", [IN, B], bf16, kind="ExternalInput").ap()
    ciw = nc.dram_tensor("ciw", [128, IN // 16], i16, kind="ExternalInput").ap()
    ci = nc.dram_tensor("ci", [128, NT], i32, kind="ExternalInput").ap()
    wq8 = nc.dram_tensor("wq8", [O_SLAB, IN], i8, kind="ExternalInput").ap()
    sT = nc.dram_tensor("sT", [OB, 128, NT], f32, kind="ExternalInput").ap()
    zT = nc.dram_tensor("zT", [OB, 128, NT], f32, kind="ExternalInput").ap()
    bias = nc.dram_tensor("bias", [1, O_SLAB], f32, kind="ExternalInput").ap()
    y = nc.dram_tensor("y", [B, O_SLAB], f32, kind="ExternalOutput").ap()

    with tile.TileContext(nc) as tc:
        for _ in range(n_bodies):
            _kernel_body(tc, xT, ciw, ci, wq8, sT, zT, bias, y,
                         bass=bass, mybir=mybir, tile=tile)

    nc.compile()
    return nc


def _kernel_body(tc, xT, ciw, ci, wq8, sT, zT, bias, y, *, bass, mybir, tile):
    from contextlib import ExitStack

    nc = tc.nc
    f32 = mybir.dt.float32
    bf16 = mybir.dt.bfloat16
    i8 = mybir.dt.int8
    i16 = mybir.dt.int16
    Alu = mybir.AluOpType
    Act = mybir.ActivationFunctionType

    no_gather = bool(os.environ.get("KERNEL_NO_GATHER"))
    no_ts = bool(os.environ.get("KERNEL_NO_TS"))
    no_xpose = bool(os.environ.get("KERNEL_NO_XPOSE"))
    no_wload = bool(os.environ.get("KERNEL_NO_WLOAD"))
    no_mm = bool(os.environ.get("KERNEL_NO_MM"))
    old_gather = bool(os.environ.get("KERNEL_OLD_GATHER"))

    with ExitStack() as ctx:
        const = ctx.enter_context(tc.tile_pool(name="const", bufs=1))
        xstage = ctx.enter_context(tc.tile_pool(name="xstage", bufs=3))
        wqpool = ctx.enter_context(tc.tile_pool(name="wq", bufs=4))
        wnpool = ctx.enter_context(tc.tile_pool(name="wn", bufs=4))
        wtpool = ctx.enter_context(tc.tile_pool(name="wt", bufs=2))
        ypool = ctx.enter_context(tc.tile_pool(name="yout", bufs=4))
        psum = ctx.enter_context(tc.tile_pool(name="psum", bufs=1, space="PSUM"))

        # --- constants ---
        ciw_sb = const.tile([128, IN // 16], i16, tag="ciw")
        nc.sync.dma_start(out=ciw_sb, in_=ciw)
        ci_sb = None
        if old_gather:
            i32 = mybir.dt.int32
            ci_sb = const.tile([128, NT], i32, tag="ci")
            nc.sync.dma_start(out=ci_sb, in_=ci)

        sT_sb = const.tile([128, OB, NT], f32, tag="sT")
        zT_sb = const.tile([128, OB, NT], f32, tag="zT")
        for ob in range(OB):
            nc.sync.dma_start(out=sT_sb[:, ob, :], in_=sT[ob])
            nc.sync.dma_start(out=zT_sb[:, ob, :], in_=zT[ob])
        # ACT dequant bias column: -(z * s) per (o, tile)
        nzs = const.tile([128, OB, NT], f32, tag="nzs")
        nc.vector.tensor_tensor(
            out=nzs.rearrange("p a b -> p (a b)"),
            in0=zT_sb.rearrange("p a b -> p (a b)"),
            in1=sT_sb.rearrange("p a b -> p (a b)"),
            op=Alu.mult,
        )
        nc.vector.tensor_scalar(
            out=nzs.rearrange("p a b -> p (a b)"),
            in0=nzs.rearrange("p a b -> p (a b)"),
            scalar1=-1.0, scalar2=None, op0=Alu.mult,
        )

        ones = const.tile([128, 128], bf16, tag="ones")
        nc.vector.memset(ones, 1.0)

        wbias = const.tile([128, O_SLAB], bf16, tag="wbias")
        nc.vector.memset(wbias, 0.0)
        bias_f = const.tile([1, O_SLAB], f32, tag="biasf")
        nc.sync.dma_start(out=bias_f, in_=bias)
        nc.vector.tensor_copy(wbias[0:1, :], bias_f)

        # ablation-mode standin tiles (allocated once; keep pipeline shape)
        if no_gather:
            xb_const = const.tile([128, 16, B], bf16, tag="xbc")
            nc.vector.memset(xb_const.rearrange("p a b -> p (a b)"), 0.5)
        if no_wload:
            wq_const = const.tile([128, 2048], i8, tag="wqc")
            nc.vector.memset(wq_const, 3)
        if no_ts:
            wn_const = const.tile([128, 16, OB // 2, 128], bf16, tag="wnc")
            nc.vector.memset(wn_const.rearrange("p a b c -> p (a b c)"), 0.25)
        if no_xpose:
            wt_const = const.tile([128, 2, 16, OB // 2, 128], bf16, tag="wtc")
            nc.vector.memset(wt_const.rearrange("p a b c d -> p (a b c d)"), 0.25)

        # PSUM accumulation groups: [b-block][o-half]
        ps = [[psum.tile([128, 512], f32, tag=f"ps{bb}{oc}", name=f"ps{bb}{oc}")
               for oc in range(2)] for bb in range(2)]

        ts_idx = 0
        n_dve = max(0, min(16, int(round(TS_DVE_SHARE * 16))))

        for ci_, (k_off, k_len) in enumerate(CHUNK_PLAN):
            tpc = k_len // 128
            # ---- x path: batched row-gather of bf16 xT rows ----
            if no_gather:
                xb = xb_const
            else:
                xb = xstage.tile([128, tpc, B], bf16, tag="xb", name=f"xb{ci_}")
                if old_gather:
                    for t in range(tpc):
                        kt = k_off // 128 + t
                        nc.gpsimd.indirect_dma_start(
                            out=xb[:, t, :],
                            out_offset=None,
                            in_=xT,
                            in_offset=bass.IndirectOffsetOnAxis(
                                ap=ci_sb[:, kt:kt + 1], axis=0),
                        )
                else:
                    s_off = k_off // 16
                    nc.gpsimd.dma_gather(
                        xb,
                        xT,
                        ciw_sb[:, s_off:s_off + k_len // 16],
                        k_len,
                        k_len,
                        B,
                    )

            # ---- W path: int8 load, dequant (DVE/ACT split), transpose ----
            # wt layout: [k-in-tile 128, oc, t, ob-in-half, o-in-block 128]
            if no_xpose:
                wt = wt_const
            else:
                wt = wtpool.tile([128, 2, tpc, OB // 2, 128], bf16, tag="wt",
                                 name=f"wt{ci_}")
            wnh = None
            if not no_ts:
                wnh = [wnpool.tile([128, tpc, OB // 2, 128], bf16, tag="wn",
                                   name=f"wn{ci_}h{h}") for h in range(2)]
            for ob in range(OB):
                if no_wload:
                    wq = wq_const
                else:
                    wq = wqpool.tile([128, k_len], i8, tag="wq",
                                     name=f"wq{ci_}o{ob}")
                    nc.sync.dma_start(
                        out=wq,
                        in_=wq8[ob * 128:(ob + 1) * 128, k_off:k_off + k_len])
                if no_ts:
                    continue
                wn, obh = wnh[ob // (OB // 2)], ob % (OB // 2)
                for t in range(tpc):
                    kt = k_off // 128 + t
                    if ts_idx % 16 < n_dve:
                        nc.vector.tensor_scalar(
                            out=wn[:, t, obh, :],
                            in0=wq[:, t * 128:(t + 1) * 128],
                            scalar1=zT_sb[:, ob, kt:kt + 1],
                            scalar2=sT_sb[:, ob, kt:kt + 1],
                            op0=Alu.subtract,
                            op1=Alu.mult,
                        )
                    else:
                        nc.scalar.activation(
                            out=wn[:, t, obh, :],
                            in_=wq[:, t * 128:(t + 1) * 128],
                            func=Act.Identity,
                            bias=nzs[:, ob, kt:kt + 1],
                            scale=sT_sb[:, ob, kt:kt + 1],
                        )
                    ts_idx += 1
            if not no_xpose:
                # both halves on the ACT ring (same-ring transposes are safe;
                # cross-ring split corrupted on HW) -- oc=0 MMs can start
                # after the first half lands
                for h in range(2):
                    src_t = (wnh[h] if wnh is not None
                             else wn_const[:, :tpc, :, :])
                    nc.scalar.dma_start_transpose(
                        wt[:, h, :, :, :],
                        src_t.rearrange("p a b c -> p (a b c)"))

            # ---- matmuls: accumulate y over this chunk's k-tiles ----
            if no_mm:
                continue
            for oc in range(2):
                for t in range(tpc):
                    kt = k_off // 128 + t
                    for bb in range(2):
                        lhsT = xb[:, t, bb * 128:(bb + 1) * 128]
                        rhs = wt[:, oc, t, :, :]
                        nc.tensor.matmul(
                            ps[bb][oc][:, :],
                            lhsT,
                            rhs,
                            start=(kt == 0),
                            stop=False,
                        )

        # ---- per-half epilogue: bias matmul closes the group, then evac ----
        for oc in range(2):
            for bb in range(2):
                nc.tensor.matmul(
                    ps[bb][oc][:, :],
                    ones,
                    wbias[:, oc * 512:(oc + 1) * 512],
                    start=no_mm,
                    stop=True,
                )
                ysb = ypool.tile([128, 512], f32, tag="ysb")
                nc.vector.tensor_copy(ysb, ps[bb][oc][:, :])
                nc.sync.dma_start(
                    out=y[bb * 128:(bb + 1) * 128, oc * 512:(oc + 1) * 512],
                    in_=ysb,
                )


def get_program():
    global _PROGRAM
    if _PROGRAM is None:
        _PROGRAM = _build_program()
    return _PROGRAM


def make_in_maps(x, W_high_q, W_low_q, scales_high, zeros_high,
                 scales_low, zeros_low, bias, col_indices):
    """Host-side sharding / layout prep. Returns per-core input dicts."""
    import ml_dtypes

    x = np.asarray(x)
    xT = np.ascontiguousarray(
        x.T.astype(ml_dtypes.bfloat16, copy=False))          # [IN, B] bf16

    # dma_gather wrapped index layout: idx i lives at [i % 16, i // 16]
    ci16 = np.asarray(col_indices).astype(np.int16, copy=False)
    ciw = np.zeros((128, IN // 16), dtype=np.int16)
    ciw[:16, :] = ci16.reshape(IN // 16, 16).T
    ci_nat = np.ascontiguousarray(
        np.asarray(col_indices).astype(np.int32, copy=False)
        .reshape(NT, 128).T)                                 # [128, NT]

    wq_all = np.concatenate(
        [np.asarray(W_high_q), np.asarray(W_low_q)], axis=1
    ).astype(np.int8)                                        # [OUT, IN] int8

    s_all = np.concatenate(
        [np.asarray(scales_high, dtype=np.float32),
         np.asarray(scales_low, dtype=np.float32)], axis=0)   # [NT, OUT]
    z_all = np.concatenate(
        [np.asarray(zeros_high, dtype=np.float32),
         np.asarray(zeros_low, dtype=np.float32)], axis=0)    # [NT, OUT]
    sT_full = np.ascontiguousarray(s_all.T)                   # [OUT, NT]
    zT_full = np.ascontiguousarray(z_all.T)                   # [OUT, NT]

    bias = np.asarray(bias, dtype=np.float32)

    in_maps = []
    for c in range(N_CORES):
        sl = slice(c * O_SLAB, (c + 1) * O_SLAB)
        in_maps.append({
            "xT": xT,
            "ciw": ciw,
            "ci": ci_nat,
            "wq8": np.ascontiguousarray(wq_all[sl]),
            "sT": np.ascontiguousarray(sT_full[sl].reshape(OB, 128, NT)),
            "zT": np.ascontiguousarray(zT_full[sl].reshape(OB, 128, NT)),
            "bias": np.ascontiguousarray(bias[sl].reshape(1, O_SLAB)),
        })
    return in_maps


def run_on_device(in_maps):
    from concourse.bass_utils import run_bass_kernel_spmd
    nc = get_program()
    res = run_bass_kernel_spmd(nc, in_maps, list(range(N_CORES)))
    out = np.concatenate(
        [res.results[c]["y"] for c in range(N_CORES)], axis=1)
    return np.ascontiguousarray(out.astype(np.float32, copy=False))


def kernel(x, W_high_q, W_low_q, scales_high, zeros_high,
           scales_low, zeros_low, bias, col_indices):
    in_maps = make_in_maps(x, W_high_q, W_low_q, scales_high, zeros_high,
                           scales_low, zeros_low, bias, col_indices)
    return run_on_device(in_maps)


# ---------------------------------------------------------------------------
# Benchmark path (test.py only): inputs parked on-device, jit built once,
# dispatches pipelined so the axon-tunnel round trip amortizes away.
# ---------------------------------------------------------------------------

class DeviceRunner:
    def __init__(self, in_maps, nc=None):
        import jax
        import numpy as _np
        from jax.experimental.shard_map import shard_map
        from jax.sharding import Mesh, NamedSharding, PartitionSpec
        import concourse.mybir as mybir
        from concourse.bass2jax import (
            _bass_exec_p, install_neuronx_cc_hook, partition_id_tensor)

        install_neuronx_cc_hook()
        if nc is None:
            nc = get_program()
        partition_name = (nc.partition_id_tensor.name
                          if nc.partition_id_tensor else None)

        in_names, out_names, out_avals, zero_outs = [], [], [], []
        for alloc in nc.m.functions[0].allocations:
            if not isinstance(alloc, mybir.MemoryLocationSet):
                continue
            name = alloc.memorylocations[0].name
            if alloc.kind == "ExternalInput":
                if name != partition_name:
                    in_names.append(name)
            elif alloc.kind == "ExternalOutput":
                shape = tuple(alloc.tensor_shape)
                dtype = mybir.dt.np(alloc.dtype)
                out_names.append(name)
                out_avals.append(jax.core.ShapedArray(shape, dtype))
                zero_outs.append(_np.zeros(shape, dtype))
        n_params = len(in_names)
        all_in_names = list(in_names) + list(out_names)
        if partition_name is not None:
            all_in_names.append(partition_name)

        def _body(*args):
            operands = list(args)
            if partition_name is not None:
                operands.append(partition_id_tensor())
            return tuple(_bass_exec_p.bind(
                *operands,
                out_avals=tuple(out_avals),
                in_names=tuple(all_in_names),
                out_names=tuple(out_names),
                lowering_input_output_aliases=(),
                sim_require_finite=True,
                sim_require_nnan=True,
                nc=nc,
            ))

        devices = jax.devices()[:N_CORES]
        mesh = Mesh(_np.asarray(devices), ("core",))
        spec = PartitionSpec("core")
        nin = n_params + len(zero_outs)
        self.fn = jax.jit(
            shard_map(_body, mesh=mesh,
                      in_specs=(spec,) * nin,
                      out_specs=(spec,) * len(out_names),
                      check_rep=False),
            keep_unused=True,
        )
        sharding = NamedSharding(mesh, spec)
        concat_in = [
            _np.concatenate([in_maps[c][k] for c in range(N_CORES)], axis=0)
            for k in in_names
        ]
        concat_zeros = [
            _np.zeros((N_CORES * z.shape[0], *z.shape[1:]), z.dtype)
            for z in zero_outs
        ]
        self.args = [jax.device_put(a, sharding)
                     for a in concat_in + concat_zeros]
        self.out_names = out_names
        self.out_avals = out_avals
        self._jax = jax

    def run(self):
        return self.fn(*self.args)

    def fetch(self, outs):
        import numpy as _np
        y = _np.asarray(outs[self.out_names.index("y")])
        y = y.reshape(N_CORES, B, O_SLAB)
        return _np.concatenate(list(y), axis=1)

    def bench(self, iters=20):
        import time
        jax = self._jax
        # warm
        outs = self.run()
        jax.block_until_ready(outs)
        t0 = time.perf_counter()
        last = None
        for _ in range(iters):
            last = self.run()
        jax.block_until_ready(last)
        dt = (time.perf_counter() - t0) / iters
        return dt, self.fetch(last)
